# revision 5
# baseline (speedup 1.0000x reference)
"""CrossAttnBlock kernel for 8 Trainium2 NeuronCores — fp8 DoubleRow,
merged-V version.

Sharding: data-parallel over B=8 -> one batch item per core; weights
replicated (cast host-side to fp8e4m3, pre-scaled to stay in range).

Key restructure vs baseline: (attn @ v) @ merge_w = attn @ (v @ merge_w),
and v = ctxa @ kvwv + bias, so host precomputes
  wcomb_h = kvwv_h @ merge_w_h   [D_ctx, D] per head  (x128 in fp8)
Per head on-chip: w8 = ctxaT^T @ wcomb8 (like the old v8), then
  out[n, d] += (expS_h^T @ w8) * rcol_h[n]
accumulated into delta in natural layout: no outT transpose-copies and
no per-head merge matmuls.

Softmax denominators fold into the attn-out matmul: an FD=1 column
matmul with the SAME stationary expS slices and a 128-valued fp8 ones
column produces den^T directly in [n-part, 1] layout, so no DRAM
round-trip transpose and the stationary operand is loaded once for
both matmuls.

Scale bookkeeping (weights x16 host-side where noted; W_SC=128 since
fp8e4's max finite value is 240):
  qT/kT hold 16q/16k             -> exp scale = (dh^-0.5)/256
  w8 = ctxa @ (128 wcomb)        -> attn psum = 128*(unnorm out)
  den column = 128.0             -> dn = 128*den, rcol = 1/dn
  -> delta += psum * rcol = (exp@wcomb)/den exactly.
  ffn: h-psum = 16*(ffa@w1): Silu(psum/16 + b1); ff-psum = 16*(h@w2):
  out = psum/16 + (x2 + b2).

v-bias and merge_b fold into a host-precomputed seed:
  delta_j := x_j + (merge_b + kv_bias_v @ merge_w), since softmax rows
  sum to 1.

LayerNorm: bn_stats/bn_aggr per 128-row chunk; rsqrt via Ln+Exp (same
activation-table set as the attention Exp); LN-apply + Swish fused into
one Silu activation with per-partition scale/bias (g==1/b==0 fast path).
"""

import json

import numpy as np

import concourse.bass as bass
import concourse.mybir as mybir
import concourse.tile as tile
from concourse.bass_utils import run_bass_kernel_spmd

F32 = mybir.dt.float32
BF16 = mybir.dt.bfloat16
FP8 = mybir.dt.float8e4
AF = mybir.ActivationFunctionType
ALU = mybir.AluOpType
DR = mybir.MatmulPerfMode.DoubleRow

P = 128
N = 1024          # query rows per core
M = 1024          # context rows per core
D = 512           # d_in == d_ctx == d_out
H = 8             # heads
DH = 64           # head dim (q/k)
DE = 2048         # ffn expand
KC = D // P       # 4 feature chunks
NCH = N // P      # 8 row chunks
ECH = DE // P     # 16 expand chunks
NS = 2
FD = 512
SCALE = DH ** -0.5
EXP_SC = SCALE / 256.0
SINV = 1.0 / 16.0
W_SC = 128.0      # host pre-scale on wcomb (fp8e4 max finite is 240!)
EPS = 1e-5


# --- workaround: this walrus build allows only ONE embedded sync wait per
# instruction.

def _split_multiwait_drains(bir_json: bytes) -> bytes:
    d = json.loads(bir_json)
    changed = False
    for fn in d.get("functions", []):
        for blk in fn.get("blocks", []):
            out = []
            for inst in blk.get("instructions", []):
                si = inst.get("sync_info") or {}
                waits = si.get("on_wait") or []
                if len(waits) > 1:
                    for j, w in enumerate(waits[:-1]):
                        out.append({
                            "name": f"{inst['name']}__w{j}",
                            "engine": inst["engine"],
                            "opcode": "NoOp",
                            "ins": [],
                            "outs": [],
                            "debug": inst.get("debug"),
                            "sync_info": {"on_wait": [w], "on_update": []},
                        })
                    si["on_wait"] = [waits[-1]]
                    changed = True
                out.append(inst)
            blk["instructions"] = out
    if not changed:
        return bir_json
    return json.dumps(d).encode()


def _install_compat():
    import concourse.bass_utils as bu
    import concourse.bass2jax as b2j

    if getattr(b2j, "_drain_split_installed", False):
        return
    orig = bu.compile_bir_kernel

    def patched(bir_json, tmpdir, neff_name="file.neff"):
        return orig(_split_multiwait_drains(bir_json), tmpdir, neff_name)

    b2j.compile_bir_kernel = patched
    b2j._drain_split_installed = True


def _bcast_1d(t, n):
    """DRAM [n] vector -> AP broadcast to [P, n] (partition stride 0)."""
    ap = t.ap()
    return bass.AP(tensor=ap.tensor, offset=ap.offset, ap=[[0, P], ap.ap[0]])


def _build(skip_gb=True, skip_bias=True):
    nc = bass.Bass("TRN2")

    x_d = nc.dram_tensor("x", [N, D], F32, kind="ExternalInput")
    ctx_d = nc.dram_tensor("context", [M, D], F32, kind="ExternalInput")
    qw_d = nc.dram_tensor("qw8", [D, DH * H], FP8, kind="ExternalInput")
    kvwk_d = nc.dram_tensor("kvwk8", [D, DH * H], FP8, kind="ExternalInput")
    wcomb_d = nc.dram_tensor("wcomb8", [D, H * D], FP8, kind="ExternalInput")
    fw1_d = nc.dram_tensor("fw18", [D, DE], FP8, kind="ExternalInput")
    fw2_d = nc.dram_tensor("fw28", [DE, D], FP8, kind="ExternalInput")
    if not skip_bias:
        qbc_d = nc.dram_tensor("qbc16", [DH * H], F32,
                               kind="ExternalInput")
        kvbc_d = nc.dram_tensor("kvbc16", [DH * H], F32,
                                kind="ExternalInput")
        fb1_d = nc.dram_tensor("fb1v", [DE], F32, kind="ExternalInput")
    mseed_d = nc.dram_tensor("mseed", [D], F32, kind="ExternalInput")
    b2_d = nc.dram_tensor("b2v", [D], F32, kind="ExternalInput")
    if not skip_gb:
        qg_d = nc.dram_tensor("q_g", [D], F32, kind="ExternalInput")
        qb_d = nc.dram_tensor("q_b", [D], F32, kind="ExternalInput")
        kvg_d = nc.dram_tensor("kv_g", [D], F32, kind="ExternalInput")
        kvb_d = nc.dram_tensor("kv_b", [D], F32, kind="ExternalInput")
        ffg_d = nc.dram_tensor("ff_g", [D], F32, kind="ExternalInput")
        ffb_d = nc.dram_tensor("ff_b", [D], F32, kind="ExternalInput")
    out_d = nc.dram_tensor("out", [N, D], F32, kind="ExternalOutput")
    import os
    DBG = os.environ.get("K2DBG", "0") == "1"
    if DBG:
        dbg_expS = nc.dram_tensor("dbg_expS", [NCH, P, N], F32,
                                  kind="Internal")
        dbg_w8 = nc.dram_tensor("dbg_w8", [NCH, P, D], F32, kind="Internal")
        dbg_dn = nc.dram_tensor("dbg_dn", [P, NCH], F32, kind="Internal")
        dbg_ap = nc.dram_tensor("dbg_ap", [NCH, P, D], F32, kind="Internal")
        dbg_delta = nc.dram_tensor("dbg_delta", [NCH, P, D], F32,
                                   kind="Internal")

    from concourse.masks import make_identity

    with tile.TileContext(nc) as tc:
        with (
            tc.tile_pool(name="pers", bufs=1) as pers,
            tc.tile_pool(name="resid", bufs=1) as resid,
        ):
            ident = pers.tile([P, P], F32, tag="ident")
            make_identity(nc, ident)
            ident_bf = pers.tile([P, P], BF16, tag="ident_bf")
            nc.vector.tensor_copy(out=ident_bf, in_=ident)
            eps_t = pers.tile([P, 1], F32, tag="eps")
            nc.vector.memset(eps_t, EPS)
            dcol8 = pers.tile([P, 2, 1], FP8, tag="dcol")
            nc.vector.memset(dcol8, W_SC)

            delta = [
                resid.tile([P, D], F32, tag=f"delta{j}", name=f"delta{j}")
                for j in range(NCH)
            ]
            mseed_rep = resid.tile([P, D], F32, tag="mseed")
            nc.gpsimd.dma_start(mseed_rep, _bcast_1d(mseed_d, D))
            if not skip_bias:
                with nc.allow_non_contiguous_dma(reason="bias gathers"):
                    qbc_c = resid.tile([P, KC], F32, tag="qbc")
                    nc.gpsimd.dma_start(
                        qbc_c, qbc_d.ap().rearrange("(o p) -> p o", p=P))
                    kvbc_c = resid.tile([P, KC], F32, tag="kvbc")
                    nc.gpsimd.dma_start(
                        kvbc_c, kvbc_d.ap().rearrange("(o p) -> p o", p=P))
            else:
                qbc_c = kvbc_c = None

            # phase-C weights + residual2: allocated early (stack order);
            # DMAs issued mid-phase-B.
            phCw_cm = tc.tile_pool(name="phCw", bufs=1)
            tCw = phCw_cm.__enter__()
            fw1_sb = tCw.tile([P, KC, DE], FP8, tag="fw1")
            fw2_sb = tCw.tile([P, ECH, D], FP8, tag="fw2")
            fb1_c = None if skip_bias else tCw.tile([P, ECH], F32,
                                                    tag="fb1")
            b2_rep = tCw.tile([P, D], F32, tag="b2rep")
            delta2 = [
                tCw.tile([P, D], F32, tag=f"d2_{j}", name=f"d2_{j}")
                for j in range(NCH)
            ]
            mvf = tCw.tile([P, NCH, 2], F32, tag="mvf")

            # activations spanning phases A+B
            acts_cm = tc.tile_pool(name="actsAB", bufs=1)
            acts = acts_cm.__enter__()
            ctxaT = acts.tile([P, KC, M], FP8, tag="ctxaT")
            xaT = acts.tile([P, KC, N], FP8, tag="xaT")
            qT = acts.tile([P, 2, 2, N], FP8, tag="qT")
            kT = acts.tile([P, 2, 2, M], FP8, tag="kT")

            if not skip_gb:
                gb_cm = tc.tile_pool(name="gb", bufs=1)
                tgb = gb_cm.__enter__()
                qg_b = tgb.tile([P, D], F32, tag="qg")
                nc.sync.dma_start(qg_b, _bcast_1d(qg_d, D))
                qb_b = tgb.tile([P, D], F32, tag="qb")
                nc.sync.dma_start(qb_b, _bcast_1d(qb_d, D))
                kvg_b = tgb.tile([P, D], F32, tag="kvg")
                nc.sync.dma_start(kvg_b, _bcast_1d(kvg_d, D))
                kvb_b = tgb.tile([P, D], F32, tag="kvb")
                nc.sync.dma_start(kvb_b, _bcast_1d(kvb_d, D))
                ffg_b = tgb.tile([P, D], F32, tag="ffg")
                nc.sync.dma_start(ffg_b, _bcast_1d(ffg_d, D))
                ffb_b = tgb.tile([P, D], F32, tag="ffb")
                nc.sync.dma_start(ffb_b, _bcast_1d(ffb_d, D))

            # ---------------- phase A ----------------
            with (
                tc.tile_pool(name="phAres", bufs=1) as ares,
                tc.tile_pool(name="phA", bufs=3) as tA,
                tc.tile_pool(name="phAst", bufs=1) as tAs,
                tc.tile_pool(name="phAw", bufs=1) as tAw,
                tc.tile_pool(name="ptA", bufs=2, space="PSUM") as ptp,
                tc.tile_pool(name="qkA", bufs=2, space="PSUM") as qkp,
            ):
                QB = 2
                cres_h, xres_h = [], []
                qs = [nc.sync, nc.scalar, nc.gpsimd]
                for qb in range(4):
                    t = ares.tile([P, QB, D], F32, tag=f"cres{qb}",
                                  name=f"cres{qb}")
                    qs[qb % 3].dma_start(
                        t, ctx_d.ap()[qb * QB * P:(qb + 1) * QB * P, :]
                        .rearrange("(c p) d -> p c d", p=P))
                    cres_h.append(t)
                for qb in range(4):
                    t = ares.tile([P, QB, D], F32, tag=f"xres{qb}",
                                  name=f"xres{qb}")
                    qs[(qb + 1) % 3].dma_start(
                        t, x_d.ap()[qb * QB * P:(qb + 1) * QB * P, :]
                        .rearrange("(c p) d -> p c d", p=P))
                    xres_h.append(t)
                cres = [cres_h[c // QB][:, c % QB, :] for c in range(NCH)]
                xres = [xres_h[c // QB][:, c % QB, :] for c in range(NCH)]

                qw_sb = tAw.tile([P, KC, DH * H], FP8, tag="qw")
                kvwk_sb = tAw.tile([P, KC, DH * H], FP8, tag="kvwk")
                nc.scalar.dma_start(
                    kvwk_sb, kvwk_d.ap().rearrange("(o p) c -> p o c", p=P))
                nc.scalar.dma_start(
                    qw_sb, qw_d.ap().rearrange("(o p) c -> p o c", p=P))

                def stats_rs(srcs, tag):
                    mv = tAs.tile([P, NCH, 2], F32, tag=f"mv_{tag}")
                    for c in range(NCH):
                        st = tA.tile([P, 6], F32, tag="st")
                        nc.vector.bn_stats(out=st, in_=srcs[c])
                        nc.vector.bn_aggr(out=mv[:, c, :], in_=st)
                    lnv = tAs.tile([P, NCH], F32, tag=f"ln_{tag}")
                    nc.scalar.activation(
                        out=lnv, in_=mv[:, :, 1:2], func=AF.Ln, bias=eps_t)
                    rs = tAs.tile([P, NCH], F32, tag=f"rs_{tag}")
                    nc.scalar.activation(
                        out=rs, in_=lnv, func=AF.Exp, scale=-0.5)
                    bias = tAs.tile([P, NCH], F32, tag=f"bi_{tag}")
                    nc.vector.scalar_tensor_tensor(
                        out=bias, in0=mv[:, :, 0:1], scalar=-1.0, in1=rs,
                        op0=ALU.mult, op1=ALU.mult)
                    return rs, bias

                def silu_transpose(srcs, rs, bias, dstT, g_b=None,
                                   b_b=None, alt=False):
                    for c in range(NCH):
                        xa = tA.tile([P, D], BF16, tag="xa")
                        if skip_gb:
                            nc.scalar.activation(
                                out=xa, in_=srcs[c], func=AF.Silu,
                                scale=rs[:, c:c + 1], bias=bias[:, c:c + 1])
                        else:
                            xf = tA.tile([P, D], F32, tag="xf")
                            nc.vector.tensor_scalar(
                                out=xf, in0=srcs[c],
                                scalar1=rs[:, c:c + 1],
                                scalar2=bias[:, c:c + 1],
                                op0=ALU.mult, op1=ALU.add)
                            nc.gpsimd.tensor_mul(out=xf, in0=xf, in1=g_b)
                            nc.gpsimd.tensor_add(out=xf, in0=xf, in1=b_b)
                            nc.scalar.activation(out=xa, in_=xf, func=AF.Silu)
                        pt = ptp.tile([P, KC, P], BF16, tag="pt")
                        for kc in range(KC):
                            nc.tensor.transpose(
                                pt[:, kc, :], xa[:, kc * P:(kc + 1) * P],
                                ident_bf)
                        if alt and c % 2 == 1:
                            nc.scalar.activation(
                                out=dstT[:, :, c * P:(c + 1) * P], in_=pt,
                                func=AF.Copy)
                        else:
                            nc.vector.tensor_copy(
                                out=dstT[:, :, c * P:(c + 1) * P], in_=pt)

                def proj_T(w_sb, rhsT, dst, bias_c, alt=False):
                    for cc in range(KC):
                        ps = qkp.tile([P, NS, FD], F32, tag="qk")
                        # kk-outer so both ns matmuls reuse the stationary
                        # operand (halves LDWEIGHTS on hardware)
                        for kk in range(KC // 2):
                            for ns in range(NS):
                                nc.tensor.matmul(
                                    ps[:, ns, :],
                                    lhsT=(w_sb[:, 2 * kk:2 * kk + 2,
                                               cc * P:(cc + 1) * P]),
                                    rhs=(rhsT[:, 2 * kk:2 * kk + 2,
                                              ns * FD:(ns + 1) * FD]),
                                    start=(kk == 0), stop=(kk == 1),
                                    perf_mode=DR)
                        o = dst[:, cc // 2, cc % 2, :]
                        if bias_c is None:
                            if alt and cc % 2 == 1:
                                nc.scalar.activation(out=o, in_=ps,
                                                     func=AF.Copy)
                            else:
                                nc.vector.tensor_copy(out=o, in_=ps)
                        else:
                            nc.vector.tensor_scalar_add(
                                out=o, in0=ps, scalar1=bias_c[:, cc:cc + 1])

                # ctx first: unblocks kT and per-head w8 matmuls
                c_rs, c_bias = stats_rs(cres, "c")
                if skip_gb:
                    silu_transpose(cres, c_rs, c_bias, ctxaT)
                else:
                    silu_transpose(cres, c_rs, c_bias, ctxaT, kvg_b, kvb_b)
                proj_T(kvwk_sb, ctxaT, kT, kvbc_c)

                for j in range(NCH):
                    nc.gpsimd.tensor_add(
                        out=delta[j], in0=xres[j], in1=mseed_rep)
                x_rs, x_bias = stats_rs(xres, "x")
                if skip_gb:
                    silu_transpose(xres, x_rs, x_bias, xaT, alt=True)
                else:
                    silu_transpose(xres, x_rs, x_bias, xaT, qg_b, qb_b,
                                   alt=True)
                proj_T(qw_sb, xaT, qT, qbc_c, alt=True)

            # ---------------- phase B: per-head attention, pipelined -------
            ffaT = tCw.tile([P, KC, N], FP8, tag="ffaT")
            lnf = tCw.tile([P, NCH], F32, tag="lnf")
            f_rs = tCw.tile([P, NCH], F32, tag="frs")
            f_bias = tCw.tile([P, NCH], F32, tag="fbi")

            phB_cm = tc.tile_pool(name="phB", bufs=1)
            tB1 = phB_cm.__enter__()
            phBs_cm = tc.tile_pool(name="phBs", bufs=2)
            tB = phBs_cm.__enter__()
            w8s = [tB1.tile([P, NCH, D], FP8, tag=f"w8_{k}",
                            name=f"w8_{k}") for k in range(2)]
            expSs = [tB1.tile([P, NCH, N], FP8, tag=f"expS_{k}",
                              name=f"expS_{k}") for k in range(2)]
            rcols = [tB1.tile([P, NCH], F32, tag=f"rcol_{k}",
                              name=f"rcol_{k}") for k in range(2)]

            def head_front(h, spp, wpp, tBw):
                wcomb_sb = tBw.tile([P, KC, D], FP8, tag="wcomb")
                nc.gpsimd.dma_start(
                    wcomb_sb,
                    wcomb_d.ap()[:, h * D:(h + 1) * D]
                    .rearrange("(o p) c -> p o c", p=P))
                return wcomb_sb

            def score_exp(h, i, spp, spc):
                expS = expSs[h % 2]
                g_h, po = h // 4, (h % 4) * 32
                sp = spp.tile([P, NS, FD], F32, tag="sp")
                for ns in range(NS):
                    nc.tensor.matmul(
                        sp[:, ns, :],
                        lhsT=kT[po:po + 32, g_h, :, i * P:(i + 1) * P],
                        rhs=qT[po:po + 32, g_h, :, ns * FD:(ns + 1) * FD],
                        start=True, stop=True, perf_mode=DR,
                        tile_position=(po, 0))
                nc.scalar.activation(
                    out=expS[:, i, :], in_=sp, func=AF.Exp, scale=EXP_SC)

            def w8_chunk(h, i, wpp, wcomb_sb, wpc):
                w8 = w8s[h % 2]
                wp = wpp.tile([P, FD], F32, tag="wp")
                for kk in range(KC // 2):
                    nc.tensor.matmul(
                        wp,
                        lhsT=ctxaT[:, 2 * kk:2 * kk + 2, i * P:(i + 1) * P],
                        rhs=wcomb_sb[:, 2 * kk:2 * kk + 2, :],
                        start=(kk == 0), stop=(kk == 1), perf_mode=DR)
                if i % 8 < 7:
                    nc.vector.tensor_copy(out=w8[:, i, :], in_=wp)
                else:
                    nc.scalar.activation(
                        out=w8[:, i, :], in_=wp, func=AF.Copy)
                if DBG and h == 0:
                    w8c = tB.tile([P, D], F32, tag="dbgw8")
                    nc.vector.tensor_copy(out=w8c, in_=w8[:, i, :])
                    nc.sync.dma_start(dbg_w8.ap()[i], w8c)
                    ec_ = tB.tile([P, N], F32, tag="dbgeS")
                    nc.vector.tensor_copy(out=ec_,
                                          in_=expSs[0][:, i, :])
                    nc.sync.dma_start(dbg_expS.ap()[i], ec_)

            def attn_out(h, j, app, dn, last):
                w8p = w8s[h % 2]
                expSp = expSs[h % 2]
                rcolp = rcols[h % 2]
                ap_ = app.tile([P, FD], F32, tag="ap")
                for ii in range(NCH // 2):
                    lhsT = expSp[:, 2 * ii:2 * ii + 2, j * P:(j + 1) * P]
                    nc.tensor.matmul(
                        ap_, lhsT=lhsT,
                        rhs=w8p[:, 2 * ii:2 * ii + 2, :],
                        start=(ii == 0), stop=(ii == 3), perf_mode=DR)
                    nc.tensor.matmul(
                        dn[:, j:j + 1], lhsT=lhsT, rhs=dcol8,
                        start=(ii == 0), stop=(ii == 3),
                        perf_mode=DR, skip_group_check=True)
                if DBG and h == 0:
                    dcp = tB.tile([P, D], F32, tag="dbgcp")
                    nc.vector.tensor_copy(out=dcp, in_=ap_)
                    nc.sync.dma_start(dbg_ap.ap()[j], dcp)
                nc.vector.reciprocal(
                    out=rcolp[:, j:j + 1], in_=dn[:, j:j + 1])
                if DBG and h == 0 and j == NCH - 1:
                    dnc = tB.tile([P, NCH], F32, tag="dbgdn")
                    nc.vector.tensor_copy(out=dnc, in_=dn)
                    nc.sync.dma_start(dbg_dn.ap(), dnc)
                if last:
                    tm = tB.tile([P, D], F32, tag="tm")
                    nc.scalar.activation(
                        out=tm, in_=ap_, func=AF.Copy,
                        scale=rcolp[:, j:j + 1])
                    nc.gpsimd.tensor_add(
                        out=delta[j], in0=delta[j], in1=tm)
                    stf = tB.tile([P, 6], F32, tag="stf")
                    nc.vector.bn_stats(out=stf, in_=delta[j])
                    nc.vector.bn_aggr(out=mvf[:, j, :], in_=stf)
                    nc.gpsimd.tensor_add(
                        out=delta2[j], in0=delta[j], in1=b2_rep)
                elif j == 3:
                    tm = tB.tile([P, D], F32, tag="tm")
                    nc.scalar.activation(
                        out=tm, in_=ap_, func=AF.Copy,
                        scale=rcolp[:, j:j + 1])
                    nc.gpsimd.tensor_add(
                        out=delta[j], in0=delta[j], in1=tm)
                else:
                    nc.vector.scalar_tensor_tensor(
                        out=delta[j], in0=ap_, scalar=rcolp[:, j:j + 1],
                        in1=delta[j], op0=ALU.mult, op1=ALU.add)
                if DBG and h == 0:
                    dcp3 = tB.tile([P, D], F32, tag="dbgcp3")
                    nc.vector.tensor_copy(out=dcp3, in_=delta[j])
                    nc.sync.dma_start(dbg_delta.ap()[j], dcp3)

            # heads 0..H-1 fronts; backs for heads 0..H-2.  Fine-grained
            # interleave per chunk so ACT exp overlaps PE attn-out without
            # PE-queue head-of-line blocking.
            with (
                tc.tile_pool(name="phBw", bufs=2) as tBw,
                tc.tile_pool(name="spB", bufs=2, space="PSUM") as spp,
                tc.tile_pool(name="wpB", bufs=1, space="PSUM") as wpp,
                tc.tile_pool(name="apB", bufs=2, space="PSUM") as app,
                tc.tile_pool(name="dnB", bufs=1, space="PSUM") as dnp,
            ):
                wcombs = [None, None]
                wcombs[0] = head_front(0, spp, wpp, tBw)
                for h in range(H):
                    if h == 2:
                        nc.sync.dma_start(
                            fw1_sb,
                            fw1_d.ap().rearrange("(o p) c -> p o c", p=P))
                        nc.sync.dma_start(
                            fw2_sb,
                            fw2_d.ap().rearrange("(o p) c -> p o c", p=P))
                        if not skip_bias:
                            with nc.allow_non_contiguous_dma(reason="bias"):
                                nc.gpsimd.dma_start(
                                    fb1_c,
                                    fb1_d.ap().rearrange("(o p) -> p o",
                                                         p=P))
                        nc.gpsimd.dma_start(b2_rep, _bcast_1d(b2_d, D))
                    wcomb_sb = wcombs[h % 2]
                    spc = [None]
                    wpc = [None]
                    if h >= 1:
                        dn = dnp.tile([P, NCH], F32, tag="dn")
                    for i in range(NCH):
                        score_exp(h, i, spp, spc)
                        w8_chunk(h, i, wpp, wcomb_sb, wpc)
                        if i == 3 and h + 1 < H:
                            # prefetch next head's merged-V weights
                            wcombs[(h + 1) % 2] = head_front(
                                h + 1, spp, wpp, tBw)
                        if h >= 1:
                            attn_out(h - 1, i, app, dn, False)

            # tail: back(H-1) + ffaT production in fresh PSUM scope
            with (
                tc.tile_pool(name="apT", bufs=2, space="PSUM") as appT,
                tc.tile_pool(name="dnT", bufs=1, space="PSUM") as dnpT,
                tc.tile_pool(name="ptT", bufs=2, space="PSUM") as ptpT,
            ):
                dn = dnpT.tile([P, NCH], F32, tag="dn")
                for j in range(NCH):
                    attn_out(H - 1, j, appT, dn, True)

                # final-LN rsqrt (exp-set, no table switch after attn exp)
                nc.scalar.activation(
                    out=lnf, in_=mvf[:, :, 1:2], func=AF.Ln, bias=eps_t)
                nc.scalar.activation(out=f_rs, in_=lnf, func=AF.Exp,
                                     scale=-0.5)
                nc.vector.scalar_tensor_tensor(
                    out=f_bias, in0=mvf[:, :, 0:1], scalar=-1.0, in1=f_rs,
                    op0=ALU.mult, op1=ALU.mult)

                for j in range(NCH):
                    fa = tB.tile([P, D], BF16, tag="fa")
                    if skip_gb:
                        nc.scalar.activation(
                            out=fa, in_=delta[j], func=AF.Silu,
                            scale=f_rs[:, j:j + 1], bias=f_bias[:, j:j + 1])
                    else:
                        xf = tB.tile([P, D], F32, tag="xf")
                        nc.vector.tensor_scalar(
                            out=xf, in0=delta[j],
                            scalar1=f_rs[:, j:j + 1],
                            scalar2=f_bias[:, j:j + 1],
                            op0=ALU.mult, op1=ALU.add)
                        nc.gpsimd.tensor_mul(out=xf, in0=xf, in1=ffg_b)
                        nc.gpsimd.tensor_add(out=xf, in0=xf, in1=ffb_b)
                        nc.scalar.activation(out=fa, in_=xf, func=AF.Silu)
                    pt = ptpT.tile([P, KC, P], BF16, tag="ptc")
                    for kc in range(KC):
                        nc.tensor.transpose(
                            pt[:, kc, :], fa[:, kc * P:(kc + 1) * P],
                            ident_bf)
                    nc.vector.tensor_copy(
                        out=ffaT[:, :, j * P:(j + 1) * P], in_=pt)

            phBs_cm.__exit__(None, None, None)
            phB_cm.__exit__(None, None, None)
            acts_cm.__exit__(None, None, None)

            # ---------------- phase C: FFN + output ----------------
            with (
                tc.tile_pool(name="phC", bufs=3) as tC,
                tc.tile_pool(name="phCl", bufs=1) as tCl,
                tc.tile_pool(name="hpC", bufs=2, space="PSUM") as hpp,
                tc.tile_pool(name="fpC", bufs=2, space="PSUM") as fpp,
            ):
                # h1T = Silu(ff_w1.T @ ffaT / 16 + b1)   [e, n] fp8
                # ffn2 runs in two half-contractions: the first half's
                # partial products accumulate into delta2 while ffn1 is
                # still streaming (fills PE/DVE during the ACT-paced
                # stretch); only the second half remains after the barrier.
                haT = tCl.tile([P, ECH, N], FP8, tag="haT")
                EH = ECH // 2

                def ffn1_ec(ec):
                    hp = hpp.tile([P, NS, FD], F32, tag="hp")
                    for kk in range(KC // 2):
                        for ns in range(NS):
                            nc.tensor.matmul(
                                hp[:, ns, :],
                                lhsT=(fw1_sb[:, 2 * kk:2 * kk + 2,
                                             ec * P:(ec + 1) * P]),
                                rhs=(ffaT[:, 2 * kk:2 * kk + 2,
                                          ns * FD:(ns + 1) * FD]),
                                start=(kk == 0), stop=(kk == 1),
                                perf_mode=DR)
                    if skip_bias:
                        nc.scalar.activation(
                            out=haT[:, ec, :], in_=hp, func=AF.Silu,
                            scale=SINV)
                    else:
                        nc.scalar.activation(
                            out=haT[:, ec, :], in_=hp, func=AF.Silu,
                            scale=SINV, bias=fb1_c[:, ec:ec + 1])

                def ffn2_half(j, half):
                    fp = fpp.tile([P, FD], F32, tag="fp")
                    for e2 in range(EH // 2):
                        ee = half * (EH // 2) + e2
                        nc.tensor.matmul(
                            fp,
                            lhsT=(haT[:, 2 * ee:2 * ee + 2,
                                      j * P:(j + 1) * P]),
                            rhs=(fw2_sb[:, 2 * ee:2 * ee + 2, :]),
                            start=(e2 == 0), stop=(e2 == EH // 2 - 1),
                            perf_mode=DR)
                    if half == 0:
                        nc.vector.scalar_tensor_tensor(
                            out=delta2[j], in0=fp, scalar=SINV,
                            in1=delta2[j], op0=ALU.mult, op1=ALU.add)
                    else:
                        ot = tC.tile([P, D], F32, tag="ot")
                        nc.vector.scalar_tensor_tensor(
                            out=ot, in0=fp, scalar=SINV, in1=delta2[j],
                            op0=ALU.mult, op1=ALU.add)
                        oq = [nc.sync, nc.scalar, nc.gpsimd][j % 3]
                        oq.dma_start(out_d.ap()[j * P:(j + 1) * P, :], ot)

                for ec in range(EH):
                    ffn1_ec(ec)
                for j in range(NCH):
                    ffn2_half(j, 0)
                    if j % 2 == 1:
                        ffn1_ec(EH + j // 2)
                for ec in range(EH + NCH // 2, ECH):
                    ffn1_ec(ec)
                for j in range(NCH):
                    ffn2_half(j, 1)

            phCw_cm.__exit__(None, None, None)
            if not skip_gb:
                gb_cm.__exit__(None, None, None)

    return nc


_CACHED = {}


def _get_nc(skip_gb, skip_bias=True):
    key = f"nc_{skip_gb}_{skip_bias}"
    if key not in _CACHED:
        _install_compat()
        _CACHED[key] = _build(skip_gb=skip_gb, skip_bias=skip_bias)
    return _CACHED[key]


def kernel(**inputs):
    import ml_dtypes
    FP8NP = ml_dtypes.float8_e4m3

    skip_gb = all(
        np.all(np.asarray(inputs[g]) == 1.0)
        and np.all(np.asarray(inputs[b]) == 0.0)
        for g, b in (("q_g", "q_b"), ("kv_g", "kv_b"), ("ff_g", "ff_b"))
    )
    kv_bias_np = np.asarray(inputs["kv_bias"], np.float32)
    skip_bias = (
        not np.any(np.asarray(inputs["q_bias"], np.float32))
        and not np.any(kv_bias_np[:DH * H])
        and not np.any(np.asarray(inputs["ff_b1"], np.float32))
    )
    nc = _get_nc(skip_gb, skip_bias)
    b = inputs["x"].shape[0]
    assert b == 8

    f32 = lambda a: np.ascontiguousarray(np.asarray(a, dtype=np.float32))
    fp8 = lambda a: np.ascontiguousarray(
        (np.asarray(a, dtype=np.float32) * 16.0).astype(FP8NP))

    kv_w = np.asarray(inputs["kv_w"], dtype=np.float32)
    kv_bias = np.asarray(inputs["kv_bias"], dtype=np.float32)
    merge_w = np.asarray(inputs["merge_w"], dtype=np.float32)

    # column permutation so each head's 64 q/k channels sit as two
    # 32-row k-tiles (enables DoubleRow on the score matmuls):
    # new[b*128 + p] = old[(4*(b//2) + p//32)*64 + 32*(b%2) + p%32]
    bb = np.arange(DH * H) // P
    pp = np.arange(DH * H) % P
    QPERM = (4 * (bb // 2) + pp // 32) * DH + 32 * (bb % 2) + pp % 32

    # merged V-through-merge weights, per head: kvwv_h @ merge_w_h
    kvwv = kv_w[:, DH * H:]                       # [D_ctx, H*D]
    wcomb = np.empty((D, H * D), np.float32)
    for h in range(H):
        wcomb[:, h * D:(h + 1) * D] = (
            kvwv[:, h * D:(h + 1) * D].astype(np.float64)
            @ merge_w[h * D:(h + 1) * D, :].astype(np.float64)
        ).astype(np.float32)

    shared = {
        "qw8": fp8(np.asarray(inputs["q_w"], np.float32)[:, QPERM]),
        "kvwk8": fp8(kv_w[:, :DH * H][:, QPERM]),
        "wcomb8": np.ascontiguousarray((wcomb * W_SC).astype(FP8NP)),
        "fw18": fp8(inputs["ff_w1"]),
        "fw28": fp8(inputs["ff_w2"]),
        "mseed": f32(np.asarray(inputs["merge_b"], np.float32)
                     + kv_bias[DH * H:] @ merge_w),
        "b2v": f32(inputs["ff_b2"]),
    }
    if not skip_bias:
        shared["qbc16"] = f32(
            np.asarray(inputs["q_bias"], np.float32)[QPERM] * 16.0)
        shared["kvbc16"] = f32(kv_bias[:DH * H][QPERM] * 16.0)
        shared["fb1v"] = f32(inputs["ff_b1"])
    if not skip_gb:
        for k in ("q_g", "q_b", "kv_g", "kv_b", "ff_g", "ff_b"):
            shared[k] = f32(inputs[k])

    in_maps = []
    for i in range(b):
        m = dict(shared)
        m["x"] = f32(inputs["x"][i])
        m["context"] = f32(inputs["context"][i])
        in_maps.append(m)
    res = run_bass_kernel_spmd(nc, in_maps, core_ids=list(range(8)))
    _CACHED["last_results"] = res
    _CACHED["last_in_maps"] = in_maps
    return np.stack([res.results[i]["out"] for i in range(8)])
